# revision 17
# baseline (speedup 1.0000x reference)
# Trainium2 Bass kernel for nn_MinLoss_15229954032079.
#
# Math: loss = sum_b sum_s dist(p[b,s], g[b,match(b,s)]) / B, where
# dist is the euclidean distance between flattened [T*D] source signals
# and match is a greedy bipartite assignment on the [S,S] distance matrix.
#
# All pairwise distances derive from the 8x8 Gram matrix of the 8 flattened
# source vectors (4 prediction sources + 4 ground-truth sources) per batch:
#   d2[s,t] = G[s,s] + G[4+t,4+t] - 2*G[s,4+t]
#
# Strategy (one NeuronCore per batch element, 8 cores; the 33.7 MB HBM
# read streams at ~430 GB/s per core, which IS the per-core HBM share —
# so runtime = preamble + stream + exposed tail, and the optimization
# targets are the head/tail, not the stream):
#   - Stream p[b], g[b] into SBUF in TAPERED windows (512, 1024x3, 256,
#     128, 128 timesteps): big windows mid-stream for minimal descriptor
#     overhead, tiny windows at the end so the exposed post-stream
#     compute tail is the compute of a 128-step window, not a 512-step
#     one. One fully contiguous DMA per (window, tensor) on the SWDGE
#     cast path (f32 HBM -> bf16 SBUF during the DMA, halving the write
#     side).
#   - DVE copies shuffle each window into a blocked bf16 layout (16
#     contiguous elements per move, both sides): block r=(ti,dg) has one
#     column group of 16 consecutive d's per source j, so every matmul
#     operand is a contiguous 128-column slice (walrus requires
#     single-free-dim matmul APs).
#   - For each 128-column block, accumulate PSUM += block^T @ block on
#     the PE (512 bf16 matmuls into one accumulation group). PSUM entry
#     (16j+u, 16j'+u) holds partial dot products of sources j,j';
#     summing the 16 u-diagonals on the host yields the exact 8x8 Gram.
#     The d=256 leftover columns accumulate into per-TI-class PSUMs
#     ([8K,8K] for windows with TI=K) so windows of different sizes
#     never mix tail packings.
#   - All PSUMs are copied into ONE [128, 248] SBUF tile and shipped
#     with a single output DMA (one descriptor-gen on the tail path
#     instead of two).
#   - Tiny [4,4] greedy matching + final scalar reduction on host.
#   - TileContext's exit sequence is patched to skip the per-semaphore
#     clear pass (each run executes a freshly loaded NEFF).

import numpy as np

B, T, S, D = 8, 4096, 4, 257
NCORES = 8
NJ = 8            # 4 pred sources + 4 gt sources

# Tapered windows: (timesteps, TI = timesteps/128). Sum of timesteps = T.
# 512-step windows for the bulk (proven clean 5.5us/window DMA pipeline);
# small windows only at the end so the exposed post-stream compute tail
# is the compute of a 128-step window. Bigger windows (1024) were tried
# and REGRESSED (116us vs 101us): they don't raise stream bandwidth but
# make 10us copy / 11.5us matmul blocks whose slot-semaphore chains
# smear the descriptor FIFO and expose a 20us serial tail.
WINDOWS = [(512, 4)] * 7 + [(128, 1)] * 4
assert sum(tw for tw, _ in WINDOWS) == T
TI_CLASSES = sorted({ti for _, ti in WINDOWS}, reverse=True)  # [8, 4, 2, 1]

# Output layout: psa [128,128] in cols 0:128, then per-TI-class tail
# psums [8K, 8K] packed at col offsets in descending K order.
OUT_COLS = 128 + sum(8 * k for k in TI_CLASSES)

_cached_nc = None


def _light_drain_and_barrier(self, tick_clock, wait_clock):
    # Replaces TileContext._drain_and_barrier: keep the drain + one
    # all-engine barrier, but skip the per-semaphore clear pass and the
    # second barrier (~6 us). Safe here because every kernel() invocation
    # executes a freshly loaded NEFF, so semaphores start from zero and
    # don't need to be restored for a re-run.
    from concourse.vector_clock import ScopedClock

    drain_inst = self.nc.sync.drain()
    wait_clock.add_sem_waits(
        drain_inst.ins, ScopedClock({None: tick_clock.global_clock})
    )
    self.nc.all_engine_barrier()
    popped = self.nc._tile_sem_poison_stack.pop()
    assert popped is self._sem_poison

def _build_nc():
    import concourse.bacc as bacc
    import concourse.tile as tile
    from concourse import mybir

    nc = bacc.Bacc("TRN2", target_bir_lowering=False, debug=False, num_swdge_queues=1)
    p_dram = nc.dram_tensor("p", [T, S, D], mybir.dt.float32, kind="ExternalInput")
    g_dram = nc.dram_tensor("g", [T, S, D], mybir.dt.float32, kind="ExternalInput")
    gram_dram = nc.dram_tensor(
        "gram", [128, 128], mybir.dt.float32, kind="ExternalOutput"
    )
    gram2_dram = nc.dram_tensor(
        "gram2", [NJ * TI_CLASSES[0], OUT_COLS - 128],
        mybir.dt.float32, kind="ExternalOutput"
    )

    orig_drain = tile.TileContext._drain_and_barrier
    tile.TileContext._drain_and_barrier = _light_drain_and_barrier

    n_body_mm = sum(ti * 16 for _, ti in WINDOWS)
    first_of_class = {}
    last_of_class = {}
    for wi, (_, ti) in enumerate(WINDOWS):
        first_of_class.setdefault(ti, wi)
        last_of_class[ti] = wi

    with tile.TileContext(nc) as tc:
        with (
            tc.tile_pool(name="slab", bufs=6) as fpool,
            tc.tile_pool(name="blk16", bufs=3) as bpool,
            tc.tile_pool(name="psum", bufs=1, space="PSUM") as ppool,
            tc.tile_pool(name="out", bufs=1) as opool,
        ):
            psa = ppool.tile([128, 128], mybir.dt.float32, tag="psa")
            psb = {}
            for k in TI_CLASSES:
                psb[k] = ppool.tile(
                    [NJ * k, NJ * k], mybir.dt.float32, name=f"psb{k}", tag=f"psb{k}"
                )

            mm_i = 0
            t0 = 0
            for wi, (tw, ti) in enumerate(WINDOWS):
                half = ti * S * D        # cols per tensor in raw HBM order
                cs = tw * D // 128       # 257*ti cols per source
                nblk = ti * 16           # full 128-col matmul blocks
                # [TW,S,D] slice -> [partition, ti, s, d]: partition p
                # covers times t0 + p*ti + ti_idx. One DMA per (window,
                # tensor): fully contiguous per partition (ti*4*257 f32).
                p_view = (
                    p_dram.ap()[t0 : t0 + tw].rearrange(
                        "(p ti) s d -> p ti s d", p=128, ti=ti
                    )
                )
                g_view = (
                    g_dram.ap()[t0 : t0 + tw].rearrange(
                        "(p ti) s d -> p ti s d", p=128, ti=ti
                    )
                )
                t0 += tw

                # slab holds the window in raw HBM order: [p-tensor | g-tensor],
                # per-partition column (ti, s, d) -> ti*1028 + s*257 + d.
                # The DMA is a plain contiguous copy that also casts
                # f32 -> bf16 (SWDGE path): the stream is read+write
                # combined bandwidth limited, so halving the write side
                # keeps the read at the HBM share.
                fsl = fpool.tile(
                    [128, 2 * half], mybir.dt.bfloat16, name=f"fsl{wi}", tag="fsl"
                )
                nc.gpsimd.dma_start(out=fsl[:, 0:half], in_=p_view)
                nc.gpsimd.dma_start(out=fsl[:, half : 2 * half], in_=g_view)

                wcols = 128 * nblk + NJ * ti
                wb = bpool.tile(
                    [128, wcols], mybir.dt.bfloat16, name=f"wb{wi}", tag="wb"
                )
                # per-source element order: q = (ti, dg, dl) — each block
                # is one ti and 16 consecutive d's per j, so copies move
                # 16-element contiguous runs on both sides. The leftover
                # d=256 gives ti tail cols per j.
                # body blocked col: (ti*16+dg)*128 + j*16 + dl
                wv = wb[:, 0 : 128 * nblk].rearrange(
                    "p (ti dg j dl) -> p j ti dg dl", ti=ti, dg=16, j=NJ, dl=16
                )
                # Split the shuffle between DVE and Activation so the
                # per-window copy latency (which gates the matmuls, and
                # through them the exposed tail) is halved. Activation's
                # strided ACTIVATE-copy is ~1.7x slower per column than
                # DVE's, so it gets 3 of the 8 sources (2 for the tiny end
                # windows where DVE's fixed costs are smaller).
                n_dve = 6 if tw == 128 else 5
                for j in range(NJ):
                    off = 0 if j < 4 else half
                    srcj = fsl[:, off : off + half].rearrange(
                        "p (ti c) -> p ti c", ti=ti
                    )[:, :, (j % 4) * D : (j % 4 + 1) * D]
                    body = srcj[:, :, 0:256].rearrange(
                        "p ti (dg dl) -> p ti dg dl", dl=16
                    )
                    tail_dst = wb[:, 128 * nblk + ti * j : 128 * nblk + ti * (j + 1)]
                    if j < n_dve:
                        nc.vector.tensor_copy(wv[:, j], body)
                        nc.vector.tensor_copy(tail_dst, srcj[:, :, 256])
                    else:
                        nc.scalar.copy(wv[:, j], body)
                        nc.scalar.copy(tail_dst, srcj[:, :, 256])

                def tail_mm():
                    tblk = wb[:, 128 * nblk : 128 * nblk + NJ * ti]
                    nc.tensor.matmul(
                        psb[ti][:],
                        tblk,
                        tblk,
                        start=(wi == first_of_class[ti]),
                        stop=(wi == last_of_class[ti]),
                    )

                # last window: tail matmul first, so the gram2 writeback
                # chain overlaps the remaining body matmuls.
                if wi == len(WINDOWS) - 1:
                    tail_mm()
                for r in range(nblk):
                    blk = wb[:, 128 * r : 128 * (r + 1)]
                    nc.tensor.matmul(
                        psa[:],
                        blk,
                        blk,
                        start=(mm_i == 0),
                        stop=(mm_i == n_body_mm - 1),
                    )
                    mm_i += 1
                if wi != len(WINDOWS) - 1:
                    tail_mm()

            # Parallel writeback: psa via DVE copy + Sync HWDGE; tail psums
            # via Activation copies + Activation HWDGE. The two chains share
            # no engine, so the exit-gating output completion is
            # max(chain) rather than their sum.
            outt = opool.tile([128, 128], mybir.dt.float32)
            outt2 = opool.tile(
                [NJ * TI_CLASSES[0], OUT_COLS - 128], mybir.dt.float32
            )
            nc.vector.tensor_copy(outt[:], psa[:])
            col = 0
            for k in TI_CLASSES:
                w = NJ * k
                nc.scalar.copy(outt2[0:w, col : col + w], psb[k][:])
                col += w
            nc.sync.dma_start(out=gram_dram.ap(), in_=outt[:])
            nc.scalar.dma_start(out=gram2_dram.ap(), in_=outt2[:])
    tile.TileContext._drain_and_barrier = orig_drain
    nc.compile()
    return nc


def _greedy_match_np(d):
    # replicate reference._greedy_match: repeated global argmin with
    # row/col masking; np.argmin matches jnp.argmin tie-breaking (first).
    s = d.shape[0]
    dm = d.astype(np.float32).copy()
    matches = np.zeros(s, np.int32)
    for _ in range(s):
        m = int(np.argmin(dm.reshape(-1)))
        r, c = divmod(m, s)
        matches[r] = c
        dm[r, :] = np.inf
        dm[:, c] = np.inf
    return matches


def _loss_from_gram(gram_list):
    total = 0.0
    for psa, gram2 in gram_list:
        # body: G8[j,j'] = sum_u psa[16j+u, 16j'+u]
        g8 = np.einsum("juku->jk", psa.reshape(8, 16, 8, 16).astype(np.float64))
        # tails: per TI-class K, psbK[kj+u, kj'+u] summed over u
        col = 0
        for k in TI_CLASSES:
            w = NJ * k
            pb = gram2[0:w, col : col + w]
            g8 += np.einsum("juku->jk", pb.reshape(8, k, 8, k).astype(np.float64))
            col += w
        pn = np.diag(g8)[:4]
        gn = np.diag(g8)[4:]
        cr = g8[:4, 4:]
        d2 = pn[:, None] + gn[None, :] - 2.0 * cr
        dists = np.sqrt(np.maximum(d2, 0.0)).astype(np.float32)
        matches = _greedy_match_np(dists)
        total += float(dists[np.arange(4), matches].astype(np.float64).sum())
    return np.float32(total / B)


def kernel(**inputs):
    global _cached_nc
    preds = np.ascontiguousarray(inputs["predictions"], dtype=np.float32)
    gts = np.ascontiguousarray(inputs["ground_truths"], dtype=np.float32)
    assert preds.shape == (B, T, S, D) and gts.shape == (B, T, S, D)

    if _cached_nc is None:
        _cached_nc = _build_nc()
    nc = _cached_nc

    from concourse.bass_utils import run_bass_kernel_spmd

    in_maps = [{"p": preds[b], "g": gts[b]} for b in range(B)]
    res = run_bass_kernel_spmd(nc, in_maps, list(range(NCORES)))
    gram_list = [
        (res.results[b]["gram"], res.results[b]["gram2"]) for b in range(B)
    ]
    return _loss_from_gram(gram_list)
